# revision 18
# baseline (speedup 1.0000x reference)
"""Distributed multi-head attention kernel for 8 Trainium2 NeuronCores — v12.

Problem: x[2,2048,768] @ Wqkv[768,2304] + bqkv -> 12-head attention -> @ Wproj + bproj.

v6 sharding: batch (2) x query-half (2) x HEAD-half (2) = 8 cores.  Each core
owns 1024 query rows and 6 of the 12 heads: it computes Q/K/V only for its 6
heads (host slices the Wqkv columns / biases per head-half, so the program
stays core-id independent), runs attention for them, and projects through its
6 heads' rows of Wproj.  The two head-half partial outputs are summed on the
HOST during unsharding (projection is linear), which removes half of the
baseline's redundant K/V projection work (~30us of PE time per core) with no
collectives (collectives pay a ~40-60us first-collective ncfw-setup +
launch-skew penalty per execution in this runtime).

Keys are permuted per-core so the core's own 1024 query tokens come first in
xT; softmax/context are permutation-invariant over keys, and this lets the
query slice be a view of xT (smaller SBUF + DMA).

Attention internals follow the baseline kernel: zero-padded per-head Q^T for
full-128-contract score matmuls (partial-array matmuls measured no faster),
packed [V_h|1] blocks whose ones-column yields the softmax denominator
through the context matmul (even local heads row 64, odd row 63), 3-deep
score/context weave paced by ScalarE exp on 2-bank PSUM groups, normalize-
late with the fast custom-DVE reciprocal broadcast via bf16 selector
matmuls, exp-table preload at t=0, and per-c-tile DMAs spread across the
sync/gpsimd/scalar queues so the PE starts at ~2us.
"""

import numpy as np
import ml_dtypes

B = 2
L = 2048
D = 768
H = 12
HL = 6             # heads per core
HD = 64
SCALE = HD ** -0.5
N_CORES = 8
LQ = 1024          # query rows per core

_CACHED = {}


def _build_nc():
    import concourse.bass as bass
    import concourse.mybir as mybir
    import concourse.tile as tile
    from concourse import bacc

    F32 = mybir.dt.float32
    BF16 = mybir.dt.bfloat16
    Alu = mybir.AluOpType
    Act = mybir.ActivationFunctionType

    nc = bacc.Bacc(target_bir_lowering=False)

    DH = HL * HD       # 384: qkv width per core
    DT = D // 128      # 6 c-tiles of the contraction dim
    KT3 = DH // 128    # 3 c-tiles of the per-core q/k dims
    LT = L // 128      # 16 key tiles
    VW = 65            # V block width per head (64 ctx + 1 ones)
    VPAD = (HL - 1) * VW + 128 + 3  # 456; head-5 window ends at 389

    xT_h = nc.declare_dram_parameter("xT", [D, L], BF16, isOutput=False)
    wqkv_h = nc.declare_dram_parameter("wqkv", [D, 3 * DH], BF16, isOutput=False)
    bqkv_h = nc.declare_dram_parameter("bqkv", [3 * DH], F32, isOutput=False)
    wp_h = nc.declare_dram_parameter("wproj2", [128, KT3, D], BF16, isOutput=False)
    sel_h = nc.declare_dram_parameter("selmat", [HL, HL * 128], BF16, isOutput=False)
    bp_h = nc.declare_dram_parameter("bproj", [D], F32, isOutput=False)
    y_h = nc.declare_dram_parameter("y", [LQ, D], F32, isOutput=True)

    with tile.TileContext(nc) as tc:
        with tc.tile_pool(name="persist", bufs=1) as pp:
            KT_sb = pp.tile([128, KT3, L], BF16)        # K^T, [c, key] layout
            QTz_sb = pp.tile([128, HL, LQ], BF16)       # Q^T per head, parity half zeroed
            V_sb = pp.tile([128, LT, VPAD], BF16)       # [V_h | ones] blocks at h*65
            OT2_sb = pp.tile([128, KT3, LQ], BF16)      # ctx^T per head PAIR
            bias_sb = pp.tile([128, 9], F32)            # q (3 kt) | k (3 kt) cols
            bv_sb = pp.tile([128, DH], F32)
            sel_sb = pp.tile([128, HL * 128], BF16)
            R16 = pp.tile([128, LQ], BF16)
            dst_sb = pp.tile([128, LQ], BF16)
            Rsb = pp.tile([128, LQ], F32)
            Dsb = pp.tile([HL, LQ], F32)
            Dall_sb = pp.tile([HL, LQ], BF16)
            junk_sb = pp.tile([128, 16], F32)

            # preload the exp activation table while the input DMAs run
            nc.vector.memset(junk_sb, 1.0)
            nc.scalar.activation(junk_sb, junk_sb, Act.Exp, scale=1.0)

            for h in range(HL):
                nc.gpsimd.memset(QTz_sb[:, h, :], 0.0)
            nc.gpsimd.memset(dst_sb, 0.0)
            nc.gpsimd.memset(Rsb, 0.0)
            nc.gpsimd.memset(R16, 0.0)
            nc.vector.memset(sel_sb, 0.0)
            nc.sync.dma_start(out=sel_sb[0:HL, :], in_=sel_h[:])
            for h in range(HL):
                nc.vector.memset(V_sb[:, :, h * VW + HD:h * VW + HD + 1], 1.0)

            nc.sync.dma_start(
                out=bias_sb[:, 0:6],
                in_=bqkv_h[0:2 * DH].rearrange("(n p) -> p n", p=128))
            bv_src = bqkv_h[2 * DH:3 * DH]
            nc.scalar.dma_start(
                out=bv_sb,
                in_=bass.AP(tensor=bv_src.tensor, offset=bv_src.offset,
                            ap=[[0, 128]] + list(bv_src.ap)),
            )
            with (
                tc.tile_pool(name="loadp", bufs=1) as lp,
                tc.tile_pool(name="ps_s", bufs=2, space="PSUM") as ps_s,
                tc.tile_pool(name="ps_o", bufs=4, space="PSUM") as ps_o,
                tc.tile_pool(name="ptp", bufs=2) as ptp,
            ):
                xT_sb = lp.tile([128, DT, L], BF16)
                wqkv_sb = lp.tile([128, DT, 3 * DH], BF16)
                wp_sb = lp.tile([128, KT3, D], BF16)

                wq_r = wqkv_h[:].rearrange("(n p) c -> p n c", p=128)
                xT_r = xT_h[:].rearrange("(n p) l -> p n l", p=128)
                # sync queue: own-query xT halves with the first Q-weight
                # c-tile between them, so qt_block(0) can start ASAP
                # critical path split across queues: Q weights on gpsimd,
                # own-query xT on sync, so qt(0) starts after ~400KB/queue
                nc.gpsimd.dma_start(
                    out=wqkv_sb[:, :, 0:128], in_=wq_r[:, :, 0:128])
                nc.sync.dma_start(
                    out=xT_sb[:, 0:3, 0:512], in_=xT_r[:, 0:3, 0:512])
                nc.sync.dma_start(
                    out=xT_sb[:, 3:6, 0:512], in_=xT_r[:, 3:6, 0:512])
                nc.sync.dma_start(
                    out=xT_sb[:, :, 512:1024], in_=xT_r[:, :, 512:1024])
                for kt in range(1, KT3):
                    nc.sync.dma_start(
                        out=wqkv_sb[:, :, kt * 128:(kt + 1) * 128],
                        in_=wq_r[:, :, kt * 128:(kt + 1) * 128])
                # gpsimd queue: K weights per c-tile, remaining xT quarters
                for kt in range(KT3):
                    nc.gpsimd.dma_start(
                        out=wqkv_sb[:, :, DH + kt * 128:DH + (kt + 1) * 128],
                        in_=wq_r[:, :, DH + kt * 128:DH + (kt + 1) * 128])
                for lc in range(2, 4):
                    nc.gpsimd.dma_start(
                        out=xT_sb[:, :, lc * 512:(lc + 1) * 512],
                        in_=xT_r[:, :, lc * 512:(lc + 1) * 512])
                # scalar queue: V weights, projection weights
                nc.scalar.dma_start(out=wqkv_sb[:, :, 2 * DH:3 * DH],
                                    in_=wq_r[:, :, 2 * DH:3 * DH])
                nc.scalar.dma_start(out=wp_sb, in_=wp_h[:])

                def qt_block(kt):
                    # Q^T c-tile over the core's 1024 queries (= xT cols
                    # 0:1024): evac halves into the zero-padded layout
                    for qh in range(2):
                        ps = ps_s.tile([128, 2, 512], F32, tag="sps")
                        for dt in range(DT):
                            nc.tensor.matmul(
                                ps[:, 0, :],
                                wqkv_sb[:, dt, kt * 128:(kt + 1) * 128],
                                xT_sb[:, dt, qh * 512:(qh + 1) * 512],
                                start=(dt == 0), stop=(dt == DT - 1),
                            )
                        nc.vector.tensor_scalar_add(
                            QTz_sb[0:64, 2 * kt, qh * 512:(qh + 1) * 512],
                            ps[0:64, 0, :], bias_sb[0:64, kt:kt + 1])
                        nc.vector.tensor_scalar_add(
                            QTz_sb[64:128, 2 * kt + 1, qh * 512:(qh + 1) * 512],
                            ps[64:128, 0, :], bias_sb[64:128, kt:kt + 1])

                def kt_block(kt, lcs=range(4)):
                    # K^T c-tile: single-op evac with per-partition bias
                    for lc in lcs:
                        ps = ps_s.tile([128, 2, 512], F32, tag="sps")
                        for dt in range(DT):
                            nc.tensor.matmul(
                                ps[:, 0, :],
                                wqkv_sb[:, dt, DH + kt * 128:DH + (kt + 1) * 128],
                                xT_sb[:, dt, lc * 512:(lc + 1) * 512],
                                start=(dt == 0), stop=(dt == DT - 1),
                            )
                        nc.vector.tensor_scalar_add(
                            KT_sb[:, kt, lc * 512:(lc + 1) * 512], ps[:, 0, :],
                            bias_sb[:, 3 + kt:4 + kt])

                def v_block(lt):
                    # all 6 local heads' V columns in one 384-wide matmul
                    ps = ps_o.tile([128, 512], F32, tag="ops")
                    for dt in range(DT):
                        nc.tensor.matmul(
                            ps[:, :384],
                            xT_sb[:, dt, lt * 128:(lt + 1) * 128],
                            wqkv_sb[:, dt, 2 * DH:3 * DH],
                            start=(dt == 0), stop=(dt == DT - 1),
                        )
                    nc.vector.tensor_tensor(
                        V_sb[:, lt, 0:390].rearrange(
                            "p (h c) -> p h c", c=VW)[:, :, 0:HD],
                        ps[:, :384].rearrange("p (h d) -> p h d", h=HL),
                        bv_sb[:, :].rearrange("p (h d) -> p h d", h=HL),
                        Alu.add,
                    )

                def score_mms(sps, h, jt):
                    for qh in range(2):
                        nc.tensor.matmul(
                            sps[:, qh, :],
                            KT_sb[:, h // 2, jt * 128:(jt + 1) * 128],
                            QTz_sb[:, h, qh * 512:(qh + 1) * 512],
                            start=True, stop=True,
                        )

                def ctx_mms(opsp, PT, h, jt, voff):
                    for qh in range(2):
                        nc.tensor.matmul(
                            opsp[qh],
                            V_sb[:, jt, voff:voff + 128],
                            PT[:, jt, qh * 512:(qh + 1) * 512],
                            start=(jt == 0), stop=(jt == LT - 1),
                            skip_group_check=True,
                        )

                def finish_out(h, opsp, eng=None):
                    # tail heads evacuate on ScalarE (idle after the last
                    # exp) so VectorE backlog doesn't hold the ps_o tiles
                    cp = (eng or nc.vector).tensor_copy if eng is None \
                        else eng.copy
                    p0 = (h % 2) * 64
                    for qh in range(2):
                        cp(
                            OT2_sb[p0:p0 + 64, h // 2, qh * 512:(qh + 1) * 512],
                            opsp[qh][p0:p0 + 64, :])
                        if h % 2 == 0:
                            cp(
                                dst_sb[64:65, qh * 512:(qh + 1) * 512],
                                opsp[qh][64:65, :])
                        else:
                            cp(
                                dst_sb[32:64, qh * 512:(qh + 1) * 512],
                                opsp[qh][32:64, :])
                    dr = 64 - (h % 2)
                    nc.sync.dma_start(
                        out=Dall_sb[h:h + 1, :], in_=dst_sb[dr:dr + 1, :])

                def s_jts(h, PT, j0, j1):
                    for jt in range(j0, j1):
                        sps = ps_s.tile([128, 2, 512], F32, tag="sps")
                        score_mms(sps, h, jt)
                        nc.scalar.activation(
                            PT[:, jt, :], sps, Act.Exp, scale=SCALE)

                def s_block(h):
                    PT = ptp.tile([128, LT, LQ], BF16, tag="PT")
                    s_jts(h, PT, 0, LT)
                    return PT

                def fused_out_s(h_out, PT_out, h_s):
                    p0 = (h_out % 2) * 64
                    voff = h_out * VW - p0
                    PT = ptp.tile([128, LT, LQ], BF16, tag="PT")
                    opsp = [ps_o.tile([128, 512], F32, tag="ops", name="opsh")
                            for _ in range(2)]
                    for jt in range(LT):
                        sps = ps_s.tile([128, 2, 512], F32, tag="sps")
                        ctx_mms(opsp, PT_out, h_out, jt, voff)
                        score_mms(sps, h_s, jt)
                        nc.scalar.activation(
                            PT[:, jt, :], sps, Act.Exp, scale=SCALE)
                    finish_out(h_out, opsp)
                    return PT

                def out_block(h, PT):
                    p0 = (h % 2) * 64
                    voff = h * VW - p0
                    opsp = [ps_o.tile([128, 512], F32, tag="ops", name="opsh")
                            for _ in range(2)]
                    for jt in range(LT):
                        ctx_mms(opsp, PT, h, jt, voff)
                    finish_out(h, opsp, eng=nc.scalar)

                def fused_last(h_out, PT_out, h_s):
                    # last block: ctx(h_out) weaves with s(h_s), and
                    # ctx(h_s) self-weaves one exp group behind s(h_s).
                    # h_s accumulates in a pinned sps-pool tile (2 banks),
                    # leaving two rotating sps buffers for the scores.
                    p0o = (h_out % 2) * 64
                    voffo = h_out * VW - p0o
                    p0s = (h_s % 2) * 64
                    voffs = h_s * VW - p0s
                    PT = ptp.tile([128, LT, LQ], BF16, tag="PT")
                    opso = [ps_o.tile([128, 512], F32, tag="ops", name="opsh")
                            for _ in range(2)]
                    ost = ps_s.tile([128, 2, 512], F32, tag="sps")
                    opss = [ost[:, 0, :], ost[:, 1, :]]
                    for jt in range(LT):
                        sps = ps_s.tile([128, 2, 512], F32, tag="sps")
                        ctx_mms(opso, PT_out, h_out, jt, voffo)
                        score_mms(sps, h_s, jt)
                        nc.scalar.activation(
                            PT[:, jt, :], sps, Act.Exp, scale=SCALE)
                        if jt > 0:
                            ctx_mms(opss, PT, h_s, jt - 1, voffs)
                    finish_out(h_out, opso, eng=nc.scalar)
                    ctx_mms(opss, PT, h_s, LT - 1, voffs)
                    finish_out(h_s, opss, eng=nc.scalar)

                def normalize(h0, h1):
                    # DVE accesses must start at a 32-aligned partition, so
                    # the elementwise ops run on [0:h1] (recompute of old
                    # rows is harmless); only the selector loop is disjoint.
                    nc.vector.tensor_copy(Dsb[0:h1, :], Dall_sb[0:h1, :])
                    nc.vector.reciprocal_approx_fast(
                        out=Rsb[0:h1, :], in_=Dsb[0:h1, :])
                    nc.vector.tensor_copy(R16[0:h1, :], Rsb[0:h1, :])
                    for h in range(h0, h1):
                        p0 = (h % 2) * 64
                        rb = ps_s.tile([128, 2, 512], F32, tag="sps")
                        for qh in range(2):
                            nc.tensor.matmul(
                                rb[:, qh, :], sel_sb[:, h * 128:(h + 1) * 128],
                                R16[:, qh * 512:(qh + 1) * 512],
                                start=True, stop=True)
                        rbf = rb[:, :, :].rearrange("p a b -> p (a b)")
                        nc.vector.tensor_tensor(
                            OT2_sb[p0:p0 + 64, h // 2, :],
                            OT2_sb[p0:p0 + 64, h // 2, :], rbf[p0:p0 + 64, :],
                            Alu.mult)

                # ---- schedule: only qt0+kt0 before the first two score
                # blocks (ScalarE starts at ~15us with a 2-head backlog that
                # covers the remaining QKV/V emission), then the weave;
                # head 5 self-weaves its context inside the last block ----
                qt_block(0)
                PT0 = ptp.tile([128, LT, LQ], BF16, tag="PT")
                for i in range(4):
                    kt_block(0, [i])
                    s_jts(0, PT0, 4 * i, 4 * i + 4)
                pending = [(0, PT0), (1, s_block(1))]
                qt_block(1)
                qt_block(2)
                kt_block(1)
                kt_block(2)
                for lt in range(LT):
                    v_block(lt)
                nexth = 2
                while pending:
                    h, PT = pending.pop(0)
                    if nexth < HL - 1:
                        pending.append((nexth, fused_out_s(h, PT, nexth)))
                        nexth += 1
                    elif nexth == HL - 1:
                        fused_last(h, PT, nexth)
                        nexth += 1
                    else:
                        out_block(h, PT)
                    if h == 1:
                        normalize(0, 2)
                    if h == 3:
                        normalize(2, 4)
                normalize(4, HL)

                # ---- projection (partial: this core's 6 heads) ----
                with tc.tile_pool(name="yp", bufs=3) as yp:
                    y_r = y_h[:].rearrange("(n p) e -> p n e", p=128)
                    for ic in range(LQ // 128):
                        for eh in range(2):
                            ps = ps_o.tile([128, 512], F32, tag="ops")
                            for pt in range(KT3):
                                nc.tensor.matmul(
                                    ps[:, :384],
                                    OT2_sb[:, pt, ic * 128:(ic + 1) * 128],
                                    wp_sb[:, pt, eh * 384:(eh + 1) * 384],
                                    start=(pt == 0), stop=(pt == KT3 - 1),
                                )
                            yt = yp.tile([128, 384], F32)
                            # bproj is folded in on the host during unshard;
                            # ScalarE (idle here) evacuates, freeing VectorE
                            nc.scalar.copy(yt, ps[:, :384])
                            dq = [nc.sync, nc.scalar, nc.gpsimd][
                                (2 * ic + eh) % 3]
                            dq.dma_start(
                                out=y_r[:, ic, eh * 384:(eh + 1) * 384], in_=yt)

    nc.finalize()
    return nc


def _get_nc():
    if "nc" not in _CACHED:
        _CACHED["nc"] = _build_nc()
    return _CACHED["nc"]


def _make_in_maps(x, Wqkv, bqkv, Wproj, bproj):
    bf16 = ml_dtypes.bfloat16
    DH = HL * HD
    x = np.asarray(x, dtype=np.float32)
    wqkv = np.asarray(Wqkv, dtype=np.float32)
    bqkv = np.asarray(bqkv, dtype=np.float32)
    wproj = np.asarray(Wproj, dtype=np.float32)
    bp32 = np.ascontiguousarray(np.asarray(bproj, dtype=np.float32))
    selmat = np.zeros((HL, HL * 128), ml_dtypes.bfloat16)
    for h in range(HL):
        selmat[h, h * 128:(h + 1) * 128] = 1.0

    xT = [np.ascontiguousarray(x[b].T.astype(bf16)) for b in range(B)]
    in_maps = []
    for c in range(N_CORES):
        b, s, hh = c // 4, (c // 2) % 2, c % 2
        # per-core weight slices: q/k/v columns of heads hh*6..hh*6+5
        d0 = hh * DH
        wq = wqkv[:, d0:d0 + DH]
        wk = wqkv[:, D + d0:D + d0 + DH]
        wv = wqkv[:, 2 * D + d0:2 * D + d0 + DH]
        wqkv_c = np.ascontiguousarray(
            np.concatenate([wq, wk, wv], axis=1).astype(bf16))
        bqkv_c = np.ascontiguousarray(np.concatenate(
            [bqkv[d0:d0 + DH], bqkv[D + d0:D + d0 + DH],
             bqkv[2 * D + d0:2 * D + d0 + DH]]))
        # wproj rows of this head-half, c-tiled
        wp_c = np.ascontiguousarray(
            wproj[d0:d0 + DH].astype(bf16)
            .reshape(DH // 128, 128, D).transpose(1, 0, 2))
        # keys permuted: own 1024 query tokens first
        xtb = xT[b]
        q0 = s * LQ
        xt_c = np.ascontiguousarray(np.concatenate(
            [xtb[:, q0:q0 + LQ], xtb[:, LQ - q0:2 * LQ - q0]], axis=1))
        in_maps.append({
            "xT": xt_c,
            "wqkv": wqkv_c,
            "bqkv": bqkv_c,
            "wproj2": wp_c,
            "bproj": bp32 if hh == 0 else np.zeros_like(bp32),
            "selmat": selmat,
        })
    return in_maps


def run(inputs, trace=False):
    """Run the SPMD kernel. Returns (full_output [2,2048,768] f32, BassKernelResults)."""
    from concourse.bass_utils import run_bass_kernel_spmd

    nc = _get_nc()
    in_maps = _make_in_maps(**inputs)
    res = run_bass_kernel_spmd(nc, in_maps, list(range(N_CORES)), trace=trace)
    bp = np.asarray(inputs["bproj"], dtype=np.float32)
    out = np.empty((B, L, D), dtype=np.float32)
    for b in range(B):
        for s in range(2):
            c0 = b * 4 + s * 2      # hh = 0
            c1 = c0 + 1             # hh = 1
            out[b, s * LQ:(s + 1) * LQ, :] = (
                res.results[c0]["y"] + res.results[c1]["y"] + bp)
    return out, res


def kernel(**inputs) -> np.ndarray:
    return run(inputs)[0]



# revision 19
# speedup vs baseline: 1.1317x; 1.1317x over previous
"""Distributed multi-head attention kernel for 8 Trainium2 NeuronCores — v12.

Problem: x[2,2048,768] @ Wqkv[768,2304] + bqkv -> 12-head attention -> @ Wproj + bproj.

v6 sharding: batch (2) x query-half (2) x HEAD-half (2) = 8 cores.  Each core
owns 1024 query rows and 6 of the 12 heads: it computes Q/K/V only for its 6
heads (host slices the Wqkv columns / biases per head-half, so the program
stays core-id independent), runs attention for them, and projects through its
6 heads' rows of Wproj.  The two head-half partial outputs are summed on the
HOST during unsharding (projection is linear), which removes half of the
baseline's redundant K/V projection work (~30us of PE time per core) with no
collectives (collectives pay a ~40-60us first-collective ncfw-setup +
launch-skew penalty per execution in this runtime).

Keys are permuted per-core so the core's own 1024 query tokens come first in
xT; softmax/context are permutation-invariant over keys, and this lets the
query slice be a view of xT (smaller SBUF + DMA).

Attention internals follow the baseline kernel: zero-padded per-head Q^T for
full-128-contract score matmuls (partial-array matmuls measured no faster),
packed [V_h|1] blocks whose ones-column yields the softmax denominator
through the context matmul (even local heads row 64, odd row 63), 3-deep
score/context weave paced by ScalarE exp on 2-bank PSUM groups, normalize-
late with the fast custom-DVE reciprocal broadcast via bf16 selector
matmuls, exp-table preload at t=0, and per-c-tile DMAs spread across the
sync/gpsimd/scalar queues so the PE starts at ~2us.
"""

import numpy as np
import ml_dtypes

B = 2
L = 2048
D = 768
H = 12
HL = 6             # heads per core
HD = 64
SCALE = HD ** -0.5
N_CORES = 8
LQ = 1024          # query rows per core

_CACHED = {}


def _build_nc():
    import concourse.bass as bass
    import concourse.mybir as mybir
    import concourse.tile as tile
    from concourse import bacc

    F32 = mybir.dt.float32
    BF16 = mybir.dt.bfloat16
    Alu = mybir.AluOpType
    Act = mybir.ActivationFunctionType

    nc = bacc.Bacc(target_bir_lowering=False)

    DH = HL * HD       # 384: qkv width per core
    DT = D // 128      # 6 c-tiles of the contraction dim
    KT3 = DH // 128    # 3 c-tiles of the per-core q/k dims
    LT = L // 128      # 16 key tiles
    VW = 65            # V block width per head (64 ctx + 1 ones)
    VPAD = (HL - 1) * VW + 128 + 3  # 456; head-5 window ends at 389

    xT_h = nc.declare_dram_parameter("xT", [D, L], BF16, isOutput=False)
    wqkv_h = nc.declare_dram_parameter("wqkv", [D, 3 * DH], BF16, isOutput=False)
    bqkv_h = nc.declare_dram_parameter("bqkv", [3 * DH], F32, isOutput=False)
    wp_h = nc.declare_dram_parameter("wproj2", [128, KT3, D], BF16, isOutput=False)
    sel_h = nc.declare_dram_parameter("selmat", [HL, HL * 128], BF16, isOutput=False)
    bp_h = nc.declare_dram_parameter("bproj", [D], F32, isOutput=False)
    y_h = nc.declare_dram_parameter("y", [LQ, D], F32, isOutput=True)

    with tile.TileContext(nc) as tc:
        with tc.tile_pool(name="persist", bufs=1) as pp:
            KT_sb = pp.tile([128, KT3, L], BF16)        # K^T, [c, key] layout
            QTz_sb = pp.tile([128, HL, LQ], BF16)       # Q^T per head, parity half zeroed
            V_sb = pp.tile([128, LT, VPAD], BF16)       # [V_h | ones] blocks at h*65
            OT2_sb = pp.tile([128, KT3, LQ], BF16)      # ctx^T per head PAIR
            bias_sb = pp.tile([128, 9], F32)            # q (3 kt) | k (3 kt) cols
            bv_sb = pp.tile([128, DH], F32)
            sel_sb = pp.tile([128, HL * 128], BF16)
            R16 = pp.tile([128, LQ], BF16)
            dst_sb = pp.tile([128, LQ], BF16)
            Rsb = pp.tile([128, LQ], F32)
            Dsb = pp.tile([HL, LQ], F32)
            Dall_sb = pp.tile([HL, LQ], BF16)
            junk_sb = pp.tile([128, 16], F32)

            # preload the exp activation table while the input DMAs run
            nc.vector.memset(junk_sb, 1.0)
            nc.scalar.activation(junk_sb, junk_sb, Act.Exp, scale=1.0)

            # memsets on VectorE (idle early) — keeping them off gpsimd
            # lets its queue issue the leading Q-weight DMA immediately
            for h in range(HL):
                nc.vector.memset(QTz_sb[:, h, :], 0.0)
            nc.vector.memset(dst_sb, 0.0)
            nc.vector.memset(Rsb, 0.0)
            nc.vector.memset(R16, 0.0)
            nc.vector.memset(sel_sb, 0.0)
            nc.sync.dma_start(out=sel_sb[0:HL, :], in_=sel_h[:])
            for h in range(HL):
                nc.vector.memset(V_sb[:, :, h * VW + HD:h * VW + HD + 1], 1.0)

            nc.sync.dma_start(
                out=bias_sb[:, 0:6],
                in_=bqkv_h[0:2 * DH].rearrange("(n p) -> p n", p=128))
            bv_src = bqkv_h[2 * DH:3 * DH]
            nc.scalar.dma_start(
                out=bv_sb,
                in_=bass.AP(tensor=bv_src.tensor, offset=bv_src.offset,
                            ap=[[0, 128]] + list(bv_src.ap)),
            )
            with (
                tc.tile_pool(name="loadp", bufs=1) as lp,
                tc.tile_pool(name="ps_s", bufs=2, space="PSUM") as ps_s,
                tc.tile_pool(name="ps_o", bufs=4, space="PSUM") as ps_o,
                tc.tile_pool(name="ptp", bufs=2) as ptp,
            ):
                xT_sb = lp.tile([128, DT, L], BF16)
                wqkv_sb = lp.tile([128, DT, 3 * DH], BF16)
                wp_sb = lp.tile([128, KT3, D], BF16)

                wq_r = wqkv_h[:].rearrange("(n p) c -> p n c", p=128)
                xT_r = xT_h[:].rearrange("(n p) l -> p n l", p=128)
                # sync queue: own-query xT halves with the first Q-weight
                # c-tile between them, so qt_block(0) can start ASAP
                # critical path split across queues: Q weights on gpsimd,
                # own-query xT on sync, so qt(0) starts after ~400KB/queue
                nc.gpsimd.dma_start(
                    out=wqkv_sb[:, :, 0:128], in_=wq_r[:, :, 0:128])
                nc.sync.dma_start(
                    out=xT_sb[:, 0:3, 0:512], in_=xT_r[:, 0:3, 0:512])
                nc.sync.dma_start(
                    out=xT_sb[:, 3:6, 0:512], in_=xT_r[:, 3:6, 0:512])
                nc.sync.dma_start(
                    out=xT_sb[:, :, 512:1024], in_=xT_r[:, :, 512:1024])
                for kt in range(1, KT3):
                    nc.sync.dma_start(
                        out=wqkv_sb[:, :, kt * 128:(kt + 1) * 128],
                        in_=wq_r[:, :, kt * 128:(kt + 1) * 128])
                # gpsimd queue: K weights per c-tile, remaining xT quarters
                for kt in range(KT3):
                    nc.gpsimd.dma_start(
                        out=wqkv_sb[:, :, DH + kt * 128:DH + (kt + 1) * 128],
                        in_=wq_r[:, :, DH + kt * 128:DH + (kt + 1) * 128])
                for lc in range(2, 4):
                    nc.gpsimd.dma_start(
                        out=xT_sb[:, :, lc * 512:(lc + 1) * 512],
                        in_=xT_r[:, :, lc * 512:(lc + 1) * 512])
                # scalar queue: V weights, projection weights
                nc.scalar.dma_start(out=wqkv_sb[:, :, 2 * DH:3 * DH],
                                    in_=wq_r[:, :, 2 * DH:3 * DH])
                nc.scalar.dma_start(out=wp_sb, in_=wp_h[:])

                def qt_block(kt):
                    # Q^T c-tile over the core's 1024 queries (= xT cols
                    # 0:1024): evac halves into the zero-padded layout
                    for qh in range(2):
                        ps = ps_s.tile([128, 2, 512], F32, tag="sps")
                        for dt in range(DT):
                            nc.tensor.matmul(
                                ps[:, 0, :],
                                wqkv_sb[:, dt, kt * 128:(kt + 1) * 128],
                                xT_sb[:, dt, qh * 512:(qh + 1) * 512],
                                start=(dt == 0), stop=(dt == DT - 1),
                            )
                        nc.vector.tensor_scalar_add(
                            QTz_sb[0:64, 2 * kt, qh * 512:(qh + 1) * 512],
                            ps[0:64, 0, :], bias_sb[0:64, kt:kt + 1])
                        nc.vector.tensor_scalar_add(
                            QTz_sb[64:128, 2 * kt + 1, qh * 512:(qh + 1) * 512],
                            ps[64:128, 0, :], bias_sb[64:128, kt:kt + 1])

                def kt_block(kt, lcs=range(4)):
                    # K^T c-tile: single-op evac with per-partition bias
                    for lc in lcs:
                        ps = ps_s.tile([128, 2, 512], F32, tag="sps")
                        for dt in range(DT):
                            nc.tensor.matmul(
                                ps[:, 0, :],
                                wqkv_sb[:, dt, DH + kt * 128:DH + (kt + 1) * 128],
                                xT_sb[:, dt, lc * 512:(lc + 1) * 512],
                                start=(dt == 0), stop=(dt == DT - 1),
                            )
                        nc.vector.tensor_scalar_add(
                            KT_sb[:, kt, lc * 512:(lc + 1) * 512], ps[:, 0, :],
                            bias_sb[:, 3 + kt:4 + kt])

                def v_block(lt):
                    # all 6 local heads' V columns in one 384-wide matmul
                    ps = ps_o.tile([128, 512], F32, tag="ops")
                    for dt in range(DT):
                        nc.tensor.matmul(
                            ps[:, :384],
                            xT_sb[:, dt, lt * 128:(lt + 1) * 128],
                            wqkv_sb[:, dt, 2 * DH:3 * DH],
                            start=(dt == 0), stop=(dt == DT - 1),
                        )
                    nc.vector.tensor_tensor(
                        V_sb[:, lt, 0:390].rearrange(
                            "p (h c) -> p h c", c=VW)[:, :, 0:HD],
                        ps[:, :384].rearrange("p (h d) -> p h d", h=HL),
                        bv_sb[:, :].rearrange("p (h d) -> p h d", h=HL),
                        Alu.add,
                    )

                def score_mms(sps, h, jt):
                    for qh in range(2):
                        nc.tensor.matmul(
                            sps[:, qh, :],
                            KT_sb[:, h // 2, jt * 128:(jt + 1) * 128],
                            QTz_sb[:, h, qh * 512:(qh + 1) * 512],
                            start=True, stop=True,
                        )

                def ctx_mms(opsp, PT, h, jt, voff):
                    for qh in range(2):
                        nc.tensor.matmul(
                            opsp[qh],
                            V_sb[:, jt, voff:voff + 128],
                            PT[:, jt, qh * 512:(qh + 1) * 512],
                            start=(jt == 0), stop=(jt == LT - 1),
                            skip_group_check=True,
                        )

                def finish_out(h, opsp, eng=None):
                    # tail heads evacuate on ScalarE (idle after the last
                    # exp) so VectorE backlog doesn't hold the ps_o tiles
                    cp = (eng or nc.vector).tensor_copy if eng is None \
                        else eng.copy
                    p0 = (h % 2) * 64
                    for qh in range(2):
                        cp(
                            OT2_sb[p0:p0 + 64, h // 2, qh * 512:(qh + 1) * 512],
                            opsp[qh][p0:p0 + 64, :])
                        if h % 2 == 0:
                            cp(
                                dst_sb[64:65, qh * 512:(qh + 1) * 512],
                                opsp[qh][64:65, :])
                        else:
                            cp(
                                dst_sb[32:64, qh * 512:(qh + 1) * 512],
                                opsp[qh][32:64, :])
                    dr = 64 - (h % 2)
                    nc.sync.dma_start(
                        out=Dall_sb[h:h + 1, :], in_=dst_sb[dr:dr + 1, :])

                def s_jts(h, PT, j0, j1):
                    for jt in range(j0, j1):
                        sps = ps_s.tile([128, 2, 512], F32, tag="sps")
                        score_mms(sps, h, jt)
                        nc.scalar.activation(
                            PT[:, jt, :], sps, Act.Exp, scale=SCALE)

                def s_block(h):
                    PT = ptp.tile([128, LT, LQ], BF16, tag="PT")
                    s_jts(h, PT, 0, LT)
                    return PT

                def fused_out_s(h_out, PT_out, h_s):
                    p0 = (h_out % 2) * 64
                    voff = h_out * VW - p0
                    PT = ptp.tile([128, LT, LQ], BF16, tag="PT")
                    opsp = [ps_o.tile([128, 512], F32, tag="ops", name="opsh")
                            for _ in range(2)]
                    for jt in range(LT):
                        sps = ps_s.tile([128, 2, 512], F32, tag="sps")
                        ctx_mms(opsp, PT_out, h_out, jt, voff)
                        score_mms(sps, h_s, jt)
                        nc.scalar.activation(
                            PT[:, jt, :], sps, Act.Exp, scale=SCALE)
                    finish_out(h_out, opsp)
                    return PT

                def out_block(h, PT):
                    p0 = (h % 2) * 64
                    voff = h * VW - p0
                    opsp = [ps_o.tile([128, 512], F32, tag="ops", name="opsh")
                            for _ in range(2)]
                    for jt in range(LT):
                        ctx_mms(opsp, PT, h, jt, voff)
                    finish_out(h, opsp, eng=nc.scalar)

                def fused_last(h_out, PT_out, h_s):
                    # last block: ctx(h_out) weaves with s(h_s), and
                    # ctx(h_s) self-weaves one exp group behind s(h_s).
                    # h_s accumulates in a pinned sps-pool tile (2 banks),
                    # leaving two rotating sps buffers for the scores.
                    p0o = (h_out % 2) * 64
                    voffo = h_out * VW - p0o
                    p0s = (h_s % 2) * 64
                    voffs = h_s * VW - p0s
                    PT = ptp.tile([128, LT, LQ], BF16, tag="PT")
                    opso = [ps_o.tile([128, 512], F32, tag="ops", name="opsh")
                            for _ in range(2)]
                    ost = ps_s.tile([128, 2, 512], F32, tag="sps")
                    opss = [ost[:, 0, :], ost[:, 1, :]]
                    for jt in range(LT):
                        sps = ps_s.tile([128, 2, 512], F32, tag="sps")
                        ctx_mms(opso, PT_out, h_out, jt, voffo)
                        score_mms(sps, h_s, jt)
                        nc.scalar.activation(
                            PT[:, jt, :], sps, Act.Exp, scale=SCALE)
                        if jt > 0:
                            ctx_mms(opss, PT, h_s, jt - 1, voffs)
                    finish_out(h_out, opso, eng=nc.scalar)
                    ctx_mms(opss, PT, h_s, LT - 1, voffs)
                    finish_out(h_s, opss, eng=nc.scalar)

                def normalize(h0, h1):
                    # DVE accesses must start at a 32-aligned partition, so
                    # the elementwise ops run on [0:h1] (recompute of old
                    # rows is harmless); only the selector loop is disjoint.
                    nc.vector.tensor_copy(Dsb[0:h1, :], Dall_sb[0:h1, :])
                    nc.vector.reciprocal_approx_fast(
                        out=Rsb[0:h1, :], in_=Dsb[0:h1, :])
                    nc.vector.tensor_copy(R16[0:h1, :], Rsb[0:h1, :])
                    for h in range(h0, h1):
                        p0 = (h % 2) * 64
                        rb = ps_s.tile([128, 2, 512], F32, tag="sps")
                        for qh in range(2):
                            nc.tensor.matmul(
                                rb[:, qh, :], sel_sb[:, h * 128:(h + 1) * 128],
                                R16[:, qh * 512:(qh + 1) * 512],
                                start=True, stop=True)
                        rbf = rb[:, :, :].rearrange("p a b -> p (a b)")
                        nc.vector.tensor_tensor(
                            OT2_sb[p0:p0 + 64, h // 2, :],
                            OT2_sb[p0:p0 + 64, h // 2, :], rbf[p0:p0 + 64, :],
                            Alu.mult)

                # ---- schedule: only qt0+kt0 before the first two score
                # blocks (ScalarE starts at ~15us with a 2-head backlog that
                # covers the remaining QKV/V emission), then the weave;
                # head 5 self-weaves its context inside the last block ----
                qt_block(0)
                PT0 = ptp.tile([128, LT, LQ], BF16, tag="PT")
                for i in range(4):
                    kt_block(0, [i])
                    s_jts(0, PT0, 4 * i, 4 * i + 4)
                pending = [(0, PT0), (1, s_block(1))]
                qt_block(1)
                qt_block(2)
                kt_block(1)
                kt_block(2)
                for lt in range(LT):
                    v_block(lt)
                nexth = 2
                while pending:
                    h, PT = pending.pop(0)
                    if nexth < HL - 1:
                        pending.append((nexth, fused_out_s(h, PT, nexth)))
                        nexth += 1
                    elif nexth == HL - 1:
                        fused_last(h, PT, nexth)
                        nexth += 1
                    else:
                        out_block(h, PT)
                    if h == 1:
                        normalize(0, 2)
                    if h == 3:
                        normalize(2, 4)
                normalize(4, HL)

                # ---- projection (partial: this core's 6 heads) ----
                with tc.tile_pool(name="yp", bufs=3) as yp:
                    y_r = y_h[:].rearrange("(n p) e -> p n e", p=128)
                    for ic in range(LQ // 128):
                        for eh in range(2):
                            ps = ps_o.tile([128, 512], F32, tag="ops")
                            for pt in range(KT3):
                                nc.tensor.matmul(
                                    ps[:, :384],
                                    OT2_sb[:, pt, ic * 128:(ic + 1) * 128],
                                    wp_sb[:, pt, eh * 384:(eh + 1) * 384],
                                    start=(pt == 0), stop=(pt == KT3 - 1),
                                )
                            yt = yp.tile([128, 384], F32)
                            # bproj is folded in on the host during unshard;
                            # ScalarE (idle here) evacuates, freeing VectorE
                            nc.scalar.copy(yt, ps[:, :384])
                            dq = [nc.sync, nc.scalar, nc.gpsimd][
                                (2 * ic + eh) % 3]
                            dq.dma_start(
                                out=y_r[:, ic, eh * 384:(eh + 1) * 384], in_=yt)

    nc.finalize()
    return nc


def _get_nc():
    if "nc" not in _CACHED:
        _CACHED["nc"] = _build_nc()
    return _CACHED["nc"]


def _make_in_maps(x, Wqkv, bqkv, Wproj, bproj):
    bf16 = ml_dtypes.bfloat16
    DH = HL * HD
    x = np.asarray(x, dtype=np.float32)
    wqkv = np.asarray(Wqkv, dtype=np.float32)
    bqkv = np.asarray(bqkv, dtype=np.float32)
    wproj = np.asarray(Wproj, dtype=np.float32)
    bp32 = np.ascontiguousarray(np.asarray(bproj, dtype=np.float32))
    selmat = np.zeros((HL, HL * 128), ml_dtypes.bfloat16)
    for h in range(HL):
        selmat[h, h * 128:(h + 1) * 128] = 1.0

    xT = [np.ascontiguousarray(x[b].T.astype(bf16)) for b in range(B)]
    in_maps = []
    for c in range(N_CORES):
        b, s, hh = c // 4, (c // 2) % 2, c % 2
        # per-core weight slices: q/k/v columns of heads hh*6..hh*6+5
        d0 = hh * DH
        wq = wqkv[:, d0:d0 + DH]
        wk = wqkv[:, D + d0:D + d0 + DH]
        wv = wqkv[:, 2 * D + d0:2 * D + d0 + DH]
        wqkv_c = np.ascontiguousarray(
            np.concatenate([wq, wk, wv], axis=1).astype(bf16))
        bqkv_c = np.ascontiguousarray(np.concatenate(
            [bqkv[d0:d0 + DH], bqkv[D + d0:D + d0 + DH],
             bqkv[2 * D + d0:2 * D + d0 + DH]]))
        # wproj rows of this head-half, c-tiled
        wp_c = np.ascontiguousarray(
            wproj[d0:d0 + DH].astype(bf16)
            .reshape(DH // 128, 128, D).transpose(1, 0, 2))
        # keys permuted: own 1024 query tokens first
        xtb = xT[b]
        q0 = s * LQ
        xt_c = np.ascontiguousarray(np.concatenate(
            [xtb[:, q0:q0 + LQ], xtb[:, LQ - q0:2 * LQ - q0]], axis=1))
        in_maps.append({
            "xT": xt_c,
            "wqkv": wqkv_c,
            "bqkv": bqkv_c,
            "wproj2": wp_c,
            "bproj": bp32 if hh == 0 else np.zeros_like(bp32),
            "selmat": selmat,
        })
    return in_maps


def run(inputs, trace=False):
    """Run the SPMD kernel. Returns (full_output [2,2048,768] f32, BassKernelResults)."""
    from concourse.bass_utils import run_bass_kernel_spmd

    nc = _get_nc()
    in_maps = _make_in_maps(**inputs)
    res = run_bass_kernel_spmd(nc, in_maps, list(range(N_CORES)), trace=trace)
    bp = np.asarray(inputs["bproj"], dtype=np.float32)
    out = np.empty((B, L, D), dtype=np.float32)
    for b in range(B):
        for s in range(2):
            c0 = b * 4 + s * 2      # hh = 0
            c1 = c0 + 1             # hh = 1
            out[b, s * LQ:(s + 1) * LQ, :] = (
                res.results[c0]["y"] + res.results[c1]["y"] + bp)
    return out, res


def kernel(**inputs) -> np.ndarray:
    return run(inputs)[0]



# revision 21
# speedup vs baseline: 1.1821x; 1.0445x over previous
"""Distributed multi-head attention kernel for 8 Trainium2 NeuronCores — v12.

Problem: x[2,2048,768] @ Wqkv[768,2304] + bqkv -> 12-head attention -> @ Wproj + bproj.

v6 sharding: batch (2) x query-half (2) x HEAD-half (2) = 8 cores.  Each core
owns 1024 query rows and 6 of the 12 heads: it computes Q/K/V only for its 6
heads (host slices the Wqkv columns / biases per head-half, so the program
stays core-id independent), runs attention for them, and projects through its
6 heads' rows of Wproj.  The two head-half partial outputs are summed on the
HOST during unsharding (projection is linear), which removes half of the
baseline's redundant K/V projection work (~30us of PE time per core) with no
collectives (collectives pay a ~40-60us first-collective ncfw-setup +
launch-skew penalty per execution in this runtime).

Keys are permuted per-core so the core's own 1024 query tokens come first in
xT; softmax/context are permutation-invariant over keys, and this lets the
query slice be a view of xT (smaller SBUF + DMA).

Attention internals follow the baseline kernel: zero-padded per-head Q^T for
full-128-contract score matmuls (partial-array matmuls measured no faster),
packed [V_h|1] blocks whose ones-column yields the softmax denominator
through the context matmul (even local heads row 64, odd row 63), 3-deep
score/context weave paced by ScalarE exp on 2-bank PSUM groups, normalize-
late with the fast custom-DVE reciprocal broadcast via bf16 selector
matmuls, exp-table preload at t=0, and per-c-tile DMAs spread across the
sync/gpsimd/scalar queues so the PE starts at ~2us.
"""

import numpy as np
import ml_dtypes

B = 2
L = 2048
D = 768
H = 12
HL = 6             # heads per core
HD = 64
SCALE = HD ** -0.5
N_CORES = 8
LQ = 1024          # query rows per core

_CACHED = {}


def _build_nc():
    import concourse.bass as bass
    import concourse.mybir as mybir
    import concourse.tile as tile
    from concourse import bacc

    F32 = mybir.dt.float32
    BF16 = mybir.dt.bfloat16
    Alu = mybir.AluOpType
    Act = mybir.ActivationFunctionType

    nc = bacc.Bacc(target_bir_lowering=False)

    DH = HL * HD       # 384: qkv width per core
    DT = D // 128      # 6 c-tiles of the contraction dim
    KT3 = DH // 128    # 3 c-tiles of the per-core q/k dims
    LT = L // 128      # 16 key tiles
    VW = 65            # V block width per head (64 ctx + 1 ones)
    VPAD = (HL - 1) * VW + 128 + 3  # 456; head-5 window ends at 389

    xT_h = nc.declare_dram_parameter("xT", [D, L], BF16, isOutput=False)
    wqkv_h = nc.declare_dram_parameter("wqkv", [D, 3 * DH], BF16, isOutput=False)
    bqkv_h = nc.declare_dram_parameter("bqkv", [3 * DH], F32, isOutput=False)
    wp_h = nc.declare_dram_parameter("wproj2", [128, KT3, D], BF16, isOutput=False)
    sel_h = nc.declare_dram_parameter("selmat", [HL, HL * 128], BF16, isOutput=False)
    bp_h = nc.declare_dram_parameter("bproj", [D], F32, isOutput=False)
    y_h = nc.declare_dram_parameter("y", [LQ, D], F32, isOutput=True)

    with tile.TileContext(nc) as tc:
        with tc.tile_pool(name="persist", bufs=1) as pp:
            KT_sb = pp.tile([128, KT3, L], BF16)        # K^T, [c, key] layout
            QTz_sb = pp.tile([128, HL, LQ], BF16)       # Q^T per head, parity half zeroed
            V_sb = pp.tile([128, LT, VPAD], BF16)       # [V_h | ones] blocks at h*65
            OT2_sb = pp.tile([128, KT3, LQ], BF16)      # ctx^T per head PAIR
            bias_sb = pp.tile([128, 9], F32)            # q (3 kt) | k (3 kt) cols
            bv_sb = pp.tile([128, DH], F32)
            sel_sb = pp.tile([128, HL * 128], BF16)
            R16 = pp.tile([128, LQ], BF16)
            dst_sb = pp.tile([128, LQ], BF16)
            Rsb = pp.tile([128, LQ], F32)
            Dsb = pp.tile([HL, LQ], F32)
            Dall_sb = pp.tile([HL, LQ], BF16)
            junk_sb = pp.tile([128, 16], F32)

            # preload the exp activation table while the input DMAs run
            nc.vector.memset(junk_sb, 1.0)
            nc.scalar.activation(junk_sb, junk_sb, Act.Exp, scale=1.0)

            for h in range(HL):
                nc.gpsimd.memset(QTz_sb[:, h, :], 0.0)
            nc.gpsimd.memset(dst_sb, 0.0)
            nc.gpsimd.memset(Rsb, 0.0)
            nc.gpsimd.memset(R16, 0.0)
            nc.vector.memset(sel_sb, 0.0)
            nc.sync.dma_start(out=sel_sb[0:HL, :], in_=sel_h[:])
            for h in range(HL):
                nc.vector.memset(V_sb[:, :, h * VW + HD:h * VW + HD + 1], 1.0)

            nc.sync.dma_start(
                out=bias_sb[:, 0:6],
                in_=bqkv_h[0:2 * DH].rearrange("(n p) -> p n", p=128))
            bv_src = bqkv_h[2 * DH:3 * DH]
            nc.scalar.dma_start(
                out=bv_sb,
                in_=bass.AP(tensor=bv_src.tensor, offset=bv_src.offset,
                            ap=[[0, 128]] + list(bv_src.ap)),
            )
            with (
                tc.tile_pool(name="loadp", bufs=1) as lp,
                tc.tile_pool(name="ps_s", bufs=2, space="PSUM") as ps_s,
                tc.tile_pool(name="ps_o", bufs=4, space="PSUM") as ps_o,
                tc.tile_pool(name="ptp", bufs=2) as ptp,
            ):
                xT_sb = lp.tile([128, DT, L], BF16)
                wqkv_sb = lp.tile([128, DT, 3 * DH], BF16)
                wp_sb = lp.tile([128, KT3, D], BF16)

                wq_r = wqkv_h[:].rearrange("(n p) c -> p n c", p=128)
                xT_r = xT_h[:].rearrange("(n p) l -> p n l", p=128)
                # sync queue: own-query xT halves with the first Q-weight
                # c-tile between them, so qt_block(0) can start ASAP
                # critical path split across queues: Q weights on gpsimd,
                # own-query xT on sync, so qt(0) starts after ~400KB/queue
                nc.gpsimd.dma_start(
                    out=wqkv_sb[:, :, 0:128], in_=wq_r[:, :, 0:128])
                nc.sync.dma_start(
                    out=xT_sb[:, 0:3, 0:512], in_=xT_r[:, 0:3, 0:512])
                nc.sync.dma_start(
                    out=xT_sb[:, 3:6, 0:512], in_=xT_r[:, 3:6, 0:512])
                nc.sync.dma_start(
                    out=xT_sb[:, :, 512:1024], in_=xT_r[:, :, 512:1024])
                for kt in range(1, KT3):
                    nc.sync.dma_start(
                        out=wqkv_sb[:, :, kt * 128:(kt + 1) * 128],
                        in_=wq_r[:, :, kt * 128:(kt + 1) * 128])
                # gpsimd queue: K weights per c-tile, remaining xT quarters
                for kt in range(KT3):
                    nc.gpsimd.dma_start(
                        out=wqkv_sb[:, :, DH + kt * 128:DH + (kt + 1) * 128],
                        in_=wq_r[:, :, DH + kt * 128:DH + (kt + 1) * 128])
                for lc in range(2, 4):
                    nc.gpsimd.dma_start(
                        out=xT_sb[:, :, lc * 512:(lc + 1) * 512],
                        in_=xT_r[:, :, lc * 512:(lc + 1) * 512])
                # scalar queue: V weights, projection weights
                nc.scalar.dma_start(out=wqkv_sb[:, :, 2 * DH:3 * DH],
                                    in_=wq_r[:, :, 2 * DH:3 * DH])
                nc.scalar.dma_start(out=wp_sb, in_=wp_h[:])

                def qt_block(kt):
                    # Q^T c-tile over the core's 1024 queries (= xT cols
                    # 0:1024): evac halves into the zero-padded layout
                    for qh in range(2):
                        ps = ps_s.tile([128, 2, 512], F32, tag="sps")
                        for dt in range(DT):
                            nc.tensor.matmul(
                                ps[:, 0, :],
                                wqkv_sb[:, dt, kt * 128:(kt + 1) * 128],
                                xT_sb[:, dt, qh * 512:(qh + 1) * 512],
                                start=(dt == 0), stop=(dt == DT - 1),
                            )
                        nc.vector.tensor_scalar_add(
                            QTz_sb[0:64, 2 * kt, qh * 512:(qh + 1) * 512],
                            ps[0:64, 0, :], bias_sb[0:64, kt:kt + 1])
                        nc.vector.tensor_scalar_add(
                            QTz_sb[64:128, 2 * kt + 1, qh * 512:(qh + 1) * 512],
                            ps[64:128, 0, :], bias_sb[64:128, kt:kt + 1])

                def kt_block(kt, lcs=range(4)):
                    # K^T c-tile: single-op evac with per-partition bias
                    for lc in lcs:
                        ps = ps_s.tile([128, 2, 512], F32, tag="sps")
                        for dt in range(DT):
                            nc.tensor.matmul(
                                ps[:, 0, :],
                                wqkv_sb[:, dt, DH + kt * 128:DH + (kt + 1) * 128],
                                xT_sb[:, dt, lc * 512:(lc + 1) * 512],
                                start=(dt == 0), stop=(dt == DT - 1),
                            )
                        nc.vector.tensor_scalar_add(
                            KT_sb[:, kt, lc * 512:(lc + 1) * 512], ps[:, 0, :],
                            bias_sb[:, 3 + kt:4 + kt])

                def v_block(lt):
                    # all 6 local heads' V columns in one 384-wide matmul
                    ps = ps_o.tile([128, 512], F32, tag="ops")
                    for dt in range(DT):
                        nc.tensor.matmul(
                            ps[:, :384],
                            xT_sb[:, dt, lt * 128:(lt + 1) * 128],
                            wqkv_sb[:, dt, 2 * DH:3 * DH],
                            start=(dt == 0), stop=(dt == DT - 1),
                        )
                    nc.vector.tensor_tensor(
                        V_sb[:, lt, 0:390].rearrange(
                            "p (h c) -> p h c", c=VW)[:, :, 0:HD],
                        ps[:, :384].rearrange("p (h d) -> p h d", h=HL),
                        bv_sb[:, :].rearrange("p (h d) -> p h d", h=HL),
                        Alu.add,
                    )

                def score_mms(sps, h, jt):
                    for qh in range(2):
                        nc.tensor.matmul(
                            sps[:, qh, :],
                            KT_sb[:, h // 2, jt * 128:(jt + 1) * 128],
                            QTz_sb[:, h, qh * 512:(qh + 1) * 512],
                            start=True, stop=True,
                        )

                def ctx_mms(opsp, PT, h, jt, voff):
                    for qh in range(2):
                        nc.tensor.matmul(
                            opsp[qh],
                            V_sb[:, jt, voff:voff + 128],
                            PT[:, jt, qh * 512:(qh + 1) * 512],
                            start=(jt == 0), stop=(jt == LT - 1),
                            skip_group_check=True,
                        )

                def finish_out(h, opsp, eng=None):
                    # tail heads evacuate on ScalarE (idle after the last
                    # exp) so VectorE backlog doesn't hold the ps_o tiles
                    cp = (eng or nc.vector).tensor_copy if eng is None \
                        else eng.copy
                    p0 = (h % 2) * 64
                    for qh in range(2):
                        cp(
                            OT2_sb[p0:p0 + 64, h // 2, qh * 512:(qh + 1) * 512],
                            opsp[qh][p0:p0 + 64, :])
                        if h % 2 == 0:
                            cp(
                                dst_sb[64:65, qh * 512:(qh + 1) * 512],
                                opsp[qh][64:65, :])
                        else:
                            cp(
                                dst_sb[32:64, qh * 512:(qh + 1) * 512],
                                opsp[qh][32:64, :])
                    dr = 64 - (h % 2)
                    nc.sync.dma_start(
                        out=Dall_sb[h:h + 1, :], in_=dst_sb[dr:dr + 1, :])

                def s_jts(h, PT, j0, j1):
                    for jt in range(j0, j1):
                        sps = ps_s.tile([128, 2, 512], F32, tag="sps")
                        score_mms(sps, h, jt)
                        nc.scalar.activation(
                            PT[:, jt, :], sps, Act.Exp, scale=SCALE)

                def s_block(h):
                    PT = ptp.tile([128, LT, LQ], BF16, tag="PT")
                    s_jts(h, PT, 0, LT)
                    return PT

                def fused_out_s(h_out, PT_out, h_s):
                    p0 = (h_out % 2) * 64
                    voff = h_out * VW - p0
                    PT = ptp.tile([128, LT, LQ], BF16, tag="PT")
                    opsp = [ps_o.tile([128, 512], F32, tag="ops", name="opsh")
                            for _ in range(2)]
                    for jt in range(LT):
                        sps = ps_s.tile([128, 2, 512], F32, tag="sps")
                        ctx_mms(opsp, PT_out, h_out, jt, voff)
                        score_mms(sps, h_s, jt)
                        nc.scalar.activation(
                            PT[:, jt, :], sps, Act.Exp, scale=SCALE)
                    finish_out(h_out, opsp)
                    return PT

                def out_block(h, PT):
                    p0 = (h % 2) * 64
                    voff = h * VW - p0
                    opsp = [ps_o.tile([128, 512], F32, tag="ops", name="opsh")
                            for _ in range(2)]
                    for jt in range(LT):
                        ctx_mms(opsp, PT, h, jt, voff)
                    finish_out(h, opsp, eng=nc.scalar)

                def fused_last(h_out, PT_out, h_s):
                    # last block: ctx(h_out) weaves with s(h_s), and
                    # ctx(h_s) self-weaves one exp group behind s(h_s).
                    # h_s accumulates in a pinned sps-pool tile (2 banks),
                    # leaving two rotating sps buffers for the scores.
                    p0o = (h_out % 2) * 64
                    voffo = h_out * VW - p0o
                    p0s = (h_s % 2) * 64
                    voffs = h_s * VW - p0s
                    PT = ptp.tile([128, LT, LQ], BF16, tag="PT")
                    opso = [ps_o.tile([128, 512], F32, tag="ops", name="opsh")
                            for _ in range(2)]
                    ost = ps_s.tile([128, 2, 512], F32, tag="sps")
                    opss = [ost[:, 0, :], ost[:, 1, :]]
                    for jt in range(LT):
                        sps = ps_s.tile([128, 2, 512], F32, tag="sps")
                        ctx_mms(opso, PT_out, h_out, jt, voffo)
                        score_mms(sps, h_s, jt)
                        nc.scalar.activation(
                            PT[:, jt, :], sps, Act.Exp, scale=SCALE)
                        if jt > 0:
                            ctx_mms(opss, PT, h_s, jt - 1, voffs)
                    finish_out(h_out, opso, eng=nc.scalar)
                    ctx_mms(opss, PT, h_s, LT - 1, voffs)
                    finish_out(h_s, opss, eng=nc.scalar)

                def normalize(h0, h1):
                    # DVE accesses must start at a 32-aligned partition, so
                    # the elementwise ops run on [0:h1] (recompute of old
                    # rows is harmless); only the selector loop is disjoint.
                    nc.vector.tensor_copy(Dsb[0:h1, :], Dall_sb[0:h1, :])
                    nc.vector.reciprocal_approx_fast(
                        out=Rsb[0:h1, :], in_=Dsb[0:h1, :])
                    nc.vector.tensor_copy(R16[0:h1, :], Rsb[0:h1, :])
                    for h in range(h0, h1):
                        p0 = (h % 2) * 64
                        rb = ps_s.tile([128, 2, 512], F32, tag="sps")
                        for qh in range(2):
                            nc.tensor.matmul(
                                rb[:, qh, :], sel_sb[:, h * 128:(h + 1) * 128],
                                R16[:, qh * 512:(qh + 1) * 512],
                                start=True, stop=True)
                        rbf = rb[:, :, :].rearrange("p a b -> p (a b)")
                        nc.vector.tensor_tensor(
                            OT2_sb[p0:p0 + 64, h // 2, :],
                            OT2_sb[p0:p0 + 64, h // 2, :], rbf[p0:p0 + 64, :],
                            Alu.mult)

                # ---- schedule: only qt0+kt0 before the first two score
                # blocks (ScalarE starts at ~15us with a 2-head backlog that
                # covers the remaining QKV/V emission), then the weave;
                # head 5 self-weaves its context inside the last block ----
                qt_block(0)
                PT0 = ptp.tile([128, LT, LQ], BF16, tag="PT")
                for i in range(4):
                    kt_block(0, [i])
                    s_jts(0, PT0, 4 * i, 4 * i + 4)
                pending = [(0, PT0), (1, s_block(1))]
                qt_block(1)
                qt_block(2)
                kt_block(1)
                kt_block(2)
                for lt in range(LT):
                    v_block(lt)
                nexth = 2
                while pending:
                    h, PT = pending.pop(0)
                    if nexth < HL - 1:
                        pending.append((nexth, fused_out_s(h, PT, nexth)))
                        nexth += 1
                    elif nexth == HL - 1:
                        fused_last(h, PT, nexth)
                        nexth += 1
                    else:
                        out_block(h, PT)
                    if h == 1:
                        normalize(0, 2)
                    if h == 3:
                        normalize(2, 4)
                normalize(4, HL)

                # ---- projection (partial: this core's 6 heads) ----
                with tc.tile_pool(name="yp", bufs=3) as yp:
                    y_r = y_h[:].rearrange("(n p) e -> p n e", p=128)
                    for ic in range(LQ // 128):
                        for eh in range(2):
                            ps = ps_o.tile([128, 512], F32, tag="ops")
                            for pt in range(KT3):
                                nc.tensor.matmul(
                                    ps[:, :384],
                                    OT2_sb[:, pt, ic * 128:(ic + 1) * 128],
                                    wp_sb[:, pt, eh * 384:(eh + 1) * 384],
                                    start=(pt == 0), stop=(pt == KT3 - 1),
                                )
                            yt = yp.tile([128, 384], F32)
                            # bproj is folded in on the host during unshard;
                            # alternate ScalarE/VectorE so evacuation keeps
                            # ahead of the PE's 3-matmul groups
                            if (2 * ic + eh) % 2 == 0:
                                nc.scalar.copy(yt, ps[:, :384])
                            else:
                                nc.vector.tensor_copy(yt, ps[:, :384])
                            dq = [nc.sync, nc.scalar, nc.gpsimd][
                                (2 * ic + eh) % 3]
                            dq.dma_start(
                                out=y_r[:, ic, eh * 384:(eh + 1) * 384], in_=yt)

    nc.finalize()
    return nc


def _get_nc():
    if "nc" not in _CACHED:
        _CACHED["nc"] = _build_nc()
    return _CACHED["nc"]


def _make_in_maps(x, Wqkv, bqkv, Wproj, bproj):
    bf16 = ml_dtypes.bfloat16
    DH = HL * HD
    x = np.asarray(x, dtype=np.float32)
    wqkv = np.asarray(Wqkv, dtype=np.float32)
    bqkv = np.asarray(bqkv, dtype=np.float32)
    wproj = np.asarray(Wproj, dtype=np.float32)
    bp32 = np.ascontiguousarray(np.asarray(bproj, dtype=np.float32))
    selmat = np.zeros((HL, HL * 128), ml_dtypes.bfloat16)
    for h in range(HL):
        selmat[h, h * 128:(h + 1) * 128] = 1.0

    xT = [np.ascontiguousarray(x[b].T.astype(bf16)) for b in range(B)]
    in_maps = []
    for c in range(N_CORES):
        b, s, hh = c // 4, (c // 2) % 2, c % 2
        # per-core weight slices: q/k/v columns of heads hh*6..hh*6+5
        d0 = hh * DH
        wq = wqkv[:, d0:d0 + DH]
        wk = wqkv[:, D + d0:D + d0 + DH]
        wv = wqkv[:, 2 * D + d0:2 * D + d0 + DH]
        wqkv_c = np.ascontiguousarray(
            np.concatenate([wq, wk, wv], axis=1).astype(bf16))
        bqkv_c = np.ascontiguousarray(np.concatenate(
            [bqkv[d0:d0 + DH], bqkv[D + d0:D + d0 + DH],
             bqkv[2 * D + d0:2 * D + d0 + DH]]))
        # wproj rows of this head-half, c-tiled
        wp_c = np.ascontiguousarray(
            wproj[d0:d0 + DH].astype(bf16)
            .reshape(DH // 128, 128, D).transpose(1, 0, 2))
        # keys permuted: own 1024 query tokens first
        xtb = xT[b]
        q0 = s * LQ
        xt_c = np.ascontiguousarray(np.concatenate(
            [xtb[:, q0:q0 + LQ], xtb[:, LQ - q0:2 * LQ - q0]], axis=1))
        in_maps.append({
            "xT": xt_c,
            "wqkv": wqkv_c,
            "bqkv": bqkv_c,
            "wproj2": wp_c,
            "bproj": bp32 if hh == 0 else np.zeros_like(bp32),
            "selmat": selmat,
        })
    return in_maps


def run(inputs, trace=False):
    """Run the SPMD kernel. Returns (full_output [2,2048,768] f32, BassKernelResults)."""
    from concourse.bass_utils import run_bass_kernel_spmd

    nc = _get_nc()
    in_maps = _make_in_maps(**inputs)
    res = run_bass_kernel_spmd(nc, in_maps, list(range(N_CORES)), trace=trace)
    bp = np.asarray(inputs["bproj"], dtype=np.float32)
    out = np.empty((B, L, D), dtype=np.float32)
    for b in range(B):
        for s in range(2):
            c0 = b * 4 + s * 2      # hh = 0
            c1 = c0 + 1             # hh = 1
            out[b, s * LQ:(s + 1) * LQ, :] = (
                res.results[c0]["y"] + res.results[c1]["y"] + bp)
    return out, res


def kernel(**inputs) -> np.ndarray:
    return run(inputs)[0]

